# revision 14
# baseline (speedup 1.0000x reference)
"""Trainium2 Bass kernel for nn_MultiHeadAttention_57337813402001.

B=4, S=2048, D=1024, H=16 heads (DH=64). 8 NeuronCores.

Sharding: core = (batch b, head-group hg); hg splits the 16 heads into two
groups of 8 (tensor parallel on the QKV projection output columns and the
output projection input rows), b is data parallel. Each core computes a
partial output projection for its 8 heads; the host sums the two partials
per batch and adds the (algebraically folded) bias terms.

Algebraic simplifications (exact in real arithmetic):
  - bk drops out of softmax (adds a per-query constant to scores).
  - bv commutes through the attention average: folded into a host-side bias
    row bv @ Wo^T added at the end.
  - softmax without max-subtraction: |scores|/sqrt(d) < ~0.7 here.

v3 dataflow (vs the 407us v2):
  - QK^T head-PAIR packing: the two heads of a pair contract only DH=64
    rows each, so their score matmuls run CONCURRENTLY in the PE array via
    row tiling (tile_position (0,0) / (64,0), disjoint row groups) -- the
    pair costs ~1 matmul slot instead of 2.  Steps become 16 pair-windows
    (j-block x head-pair) of 16 single-chunk groups; scores for both heads
    of a key chunk land in one [128, 2(head), 512] PSUM tile so the exp op
    stays N=1024.
  - Q/K path in fp8e4: projections use MatmulPerfMode.DoubleRow; QK^T runs
    plain fp8 in the natural [pair-dh, s] layout. Scores only feed
    exp(s/32); measured ~4e-3 end to end.
  - V path and everything after exp stays fp16 (fp8 there costs ~4% output
    error).
  - Software-pipelined schedule per group: [PV(w-1) | fillers | QK(w)+exp]
    with the dependency-blocking QK last so the in-order PE queue never
    idles. Filler work (V/K/Q projection chunks, prior row's output
    projection) comes from a deadline + uniform-rate scheduler.
  - PSUM: qk 2x[128,2,512] + pv 2x[128,512] + mm(fillers+bc) 2x[128,512]
    = 8 banks exactly.
"""

import os
import sys

import numpy as np

for _p in ("/opt/trn_rl_repo",):
    if _p not in sys.path and os.path.isdir(_p):
        sys.path.insert(0, _p)

B, S, D, H = 4, 2048, 1024, 16
DH = D // H          # 64
HL = H // 2          # 8 heads per core
DL = HL * DH         # 512 local hidden
P = 128
KC = D // P          # 8 d_in chunks
CC = DL // P         # 4 local d_out chunks (= head pairs)
N_CORES = 8

QK_FP8 = True        # fp8 for QK^T scores
PROJ_FP8 = True      # DoubleRow fp8 for the Q/K projections (host fp8 in)


def build_bass(s=S):
    import concourse.bass as bass  # noqa: F401
    import concourse.mybir as mybir
    from concourse import bacc
    from concourse.tile import TileContext

    dt16 = mybir.dt.float16
    f8 = mybir.dt.float8e4
    f32 = mybir.dt.float32
    AF = mybir.ActivationFunctionType
    DR = mybir.MatmulPerfMode.DoubleRow

    nsk = s // P                 # key chunks (16) = groups per window
    sqb = min(512, s)            # sq block
    nsqb = s // sqb              # 4 j rows
    sb_blk = min(512, s)
    nsb = s // sb_blk            # 4 projection s blocks
    qdt = f8 if PROJ_FP8 else dt16

    nc = bacc.Bacc()
    QT = nc.declare_dram_parameter("QT", [D, s], qdt, isOutput=False)
    KT = nc.declare_dram_parameter("KT", [D, s], qdt, isOutput=False)
    VT = nc.declare_dram_parameter("VT", [D, s], dt16, isOutput=False)
    WQT = nc.declare_dram_parameter("WQT", [D, DL], qdt, isOutput=False)
    WKT = nc.declare_dram_parameter("WKT", [D, DL], qdt, isOutput=False)
    WVT = nc.declare_dram_parameter("WVT", [D, DL], dt16, isOutput=False)
    WOT = nc.declare_dram_parameter("WOT", [DL, D], dt16, isOutput=False)
    BQ = nc.declare_dram_parameter("BQ", [P, CC], f32, isOutput=False)
    OUT = nc.declare_dram_parameter("OUT", [s, D], dt16, isOutput=True)

    with TileContext(nc) as tc:
        with (
            tc.tile_pool(name="w", bufs=1) as wp,
            tc.tile_pool(name="stq", bufs=1) as stq,
            tc.tile_pool(name="stk", bufs=1) as stk,
            tc.tile_pool(name="stv", bufs=2) as stv,
            tc.tile_pool(name="qkv", bufs=1) as qkvp,
            tc.tile_pool(name="E", bufs=2) as ep,
            tc.tile_pool(name="rc", bufs=2) as rcp,
            tc.tile_pool(name="ost", bufs=3) as ostp,
            tc.tile_pool(name="qkps", bufs=2, space="PSUM") as qkps,
            tc.tile_pool(name="pvps", bufs=2, space="PSUM") as pvps,
            tc.tile_pool(name="mmps", bufs=2, space="PSUM") as mmps,
        ):
            # --- weights / constants ---
            wq = wp.tile([P, KC, DL], qdt, tag="wq")
            wk = wp.tile([P, KC, DL], qdt, tag="wk")
            wv = wp.tile([P, KC, DL], dt16, tag="wv")
            wo = wp.tile([P, CC, D], dt16, tag="wo")
            bq = wp.tile([P, CC], f32, tag="bq")
            ones_row = wp.tile([1, DH], dt16, tag="ones")
            qst = stq.tile([P, KC, s], qdt, tag="qst")
            kst = stk.tile([P, KC, s], qdt, tag="kst")

            # DMA issue order feeds the critical path: k projection of
            # s-block b needs only wk + kst block b; q pair0 needs wq + qst
            # block 0. K path on the SP queue, Q path on the (idle) gpsimd
            # queue, V/O weights on the vector queue -- three staging chains
            # land concurrently instead of serializing on one queue.
            def stage_blk(dst, src, blk, eng):
                eng.dma_start(
                    dst[:, :, blk * sb_blk:(blk + 1) * sb_blk],
                    src[:, blk * sb_blk:(blk + 1) * sb_blk].rearrange(
                        "(kc p) ss -> p kc ss", p=P
                    ),
                )

            nc.sync.dma_start(wk, WKT[:].rearrange("(kc p) m -> p kc m", p=P))
            stage_blk(kst, KT, 0, nc.sync)
            nc.gpsimd.dma_start(wq, WQT[:].rearrange("(kc p) m -> p kc m", p=P))
            nc.gpsimd.dma_start(bq, BQ[:])
            stage_blk(qst, QT, 0, nc.gpsimd)
            for blk in range(1, nsb):
                stage_blk(kst, KT, blk, nc.sync)
            nc.scalar.dma_start(wv, WVT[:].rearrange("(kc p) m -> p kc m", p=P))
            for blk in range(1, nsb):
                stage_blk(qst, QT, blk, nc.gpsimd)
            nc.scalar.dma_start(wo, WOT[:].rearrange("(cc p) m -> p cc m", p=P))
            nc.vector.memset(ones_row, 1.0)

            # q/k fp8 in natural projection layout: [128, pair, s], partition
            # = 64*hi + dh for head 2*pair + hi. QK^T runs plain fp8 (K=64)
            # with the pair's two heads row-tiled concurrently.
            qT8 = qkvp.tile([P, CC, s], f8, tag="qT8")
            kT8 = qkvp.tile([P, CC, s], f8, tag="kT8")
            vpad = qkvp.tile([P, nsk, HL, DH + 1], dt16, tag="vpad")
            aT = qkvp.tile([P, CC, s], dt16, tag="aT")
            nc.vector.memset(vpad[:, :, :, DH], 1.0)

            scale = 1.0 / np.sqrt(np.float32(D)).item()

            # ---------- emission helpers ----------
            def proj_qk_chunk(xst, w, dst8, c, blk, bias=None):
                """One [128,512] chunk of a q/k projection (fp8 DoubleRow,
                2 d_in k-tiles per matmul), cast straight into dst8[:, c]."""
                ps = mmps.tile([P, sb_blk], f32, tag="mm")
                if PROJ_FP8:
                    for k2 in range(KC // 2):
                        nc.tensor.matmul(
                            ps,
                            lhsT=w[:, 2 * k2:2 * k2 + 2, c * P:(c + 1) * P],
                            rhs=xst[:, 2 * k2:2 * k2 + 2,
                                    blk * sb_blk:(blk + 1) * sb_blk],
                            start=(k2 == 0),
                            stop=(k2 == KC // 2 - 1),
                            perf_mode=DR,
                        )
                else:
                    for k in range(KC):
                        nc.tensor.matmul(
                            ps,
                            lhsT=w[:, k, c * P:(c + 1) * P],
                            rhs=xst[:, k, blk * sb_blk:(blk + 1) * sb_blk],
                            start=(k == 0),
                            stop=(k == KC - 1),
                        )
                dsl = dst8[:, c, blk * sb_blk:(blk + 1) * sb_blk]
                with nc.allow_low_precision(reason="fp8 q/k by design"):
                    if bias is not None:
                        nc.vector.tensor_scalar_add(
                            out=dsl, in0=ps, scalar1=bias[:, c:c + 1],
                        )
                    else:
                        nc.vector.tensor_copy(out=dsl, in_=ps)

            def v_chunk(xv, cg, li, gi):
                """V projection for pair-group cg (pairs 2cg,2cg+1), local
                chunk li of the staged block = global sk chunk gi; N=256."""
                ps = mmps.tile([P, 256], f32, tag="mm")
                for k in range(KC):
                    nc.tensor.matmul(
                        ps,
                        lhsT=xv[:, k, li * P:(li + 1) * P],
                        rhs=wv[:, k, cg * 256:(cg + 1) * 256],
                        start=(k == 0),
                        stop=(k == KC - 1),
                    )
                with nc.allow_low_precision(reason="fp16 v by design"):
                    nc.vector.tensor_copy(
                        out=vpad[:, gi, 4 * cg:4 * cg + 4, 0:DH],
                        in_=ps.rearrange("p (h d) -> p h d", d=DH),
                    )

            # ---------- filler queue (PE work interleaved into the
            # attention pipeline; ~each item <= ~1us of PE time) ----------
            def stage_v(blk):
                xv = stv.tile([P, KC, sb_blk], dt16, tag="stv")
                nc.gpsimd.dma_start(
                    xv,
                    VT[:, blk * sb_blk:(blk + 1) * sb_blk].rearrange(
                        "(kc p) ss -> p kc ss", p=P
                    ),
                )
                return xv

            def oproj_chunk(sc, db):
                ps = mmps.tile([P, 512], f32, tag="mm")
                for c in range(CC):
                    nc.tensor.matmul(
                        ps,
                        lhsT=aT[:, c, sc * P:(sc + 1) * P],
                        rhs=wo[:, c, db * 512:(db + 1) * 512],
                        start=(c == 0),
                        stop=(c == CC - 1),
                    )
                ot = ostp.tile([P, 512], dt16, tag="ost")
                with nc.allow_low_precision(reason="fp16 partial"):
                    nc.vector.tensor_copy(out=ot, in_=ps)
                nc.sync.dma_start(
                    OUT[sc * P:(sc + 1) * P, db * 512:(db + 1) * 512], ot
                )

            # ---------- preamble: only k c0 blk0 + q pair0 c0 gate QK(0,0);
            # the rest become tight-deadline fillers.
            proj_qk_chunk(kst, wk, kT8, 0, 0)
            proj_qk_chunk(qst, wq, qT8, 0, 0, bias=bq)

            # ---------- filler scheduler ----------
            # Items = (deadline_slot, release_slot, est_pe_ns, fn), emitted
            # into group slots at a uniform PE-time rate with deadline
            # forcing, so the in-order PE queue always has ready work (the
            # p-state model halves the PE clock for 3us after any idle gap).
            # slot = window * nsk + group; windows = npairs + 2.
            pairs = [(j, p) for j in range(nsqb) for p in range(CC)]
            npairs = len(pairs)                      # 16
            total_slots = (npairs + 1) * nsk
            BIG = 10 ** 9
            items = []
            v_stage = {}

            def v_item(cg, blk, li):
                def _f():
                    if blk not in v_stage or v_stage[blk][1] != (cg,):
                        v_stage[blk] = (stage_v(blk), (cg,))
                    v_chunk(v_stage[blk][0], cg, li, blk * (sb_blk // P) + li)
                gi = blk * (sb_blk // P) + li
                # PV(w-1) in window w reads vpad chunk gi at group gi//2;
                # cg first used by pair 2cg (window 2cg, PV in 2cg+1).
                dl = (1 + 2 * cg) * nsk + gi // 2 - 2
                return (max(1, dl), 0, 900, _f)

            for cg in range(2):
                for blk in range(nsb):
                    for li in range(sb_blk // P):
                        items.append(v_item(cg, blk, li))

            for c in range(CC):
                for blk in range(nsb):
                    if c == 0 and blk == 0:
                        continue
                    # kT8[:, c, blk] feeds QK of pair-window c, groups
                    # 4blk..4blk+3; release keeps early c0 blocks from
                    # popping before their staging DMAs land.
                    items.append((
                        max(1, c * nsk + 4 * blk - 4),
                        max(0, 4 * blk - 3) if c == 0 else 0, 900,
                        lambda c=c, blk=blk: proj_qk_chunk(kst, wk, kT8, c, blk),
                    ))
            for jq in range(nsqb):
                for c in range(CC):
                    if jq == 0 and c == 0:
                        continue
                    # qT8[:, c, jq] feeds QK of window 4jq + c, group 0.
                    items.append((
                        max(1, (4 * jq + c) * nsk - 12), 0, 900,
                        lambda c=c, jq=jq: proj_qk_chunk(
                            qst, wq, qT8, c, jq, bias=bq),
                    ))
            # output projection of row j: released one window after
            # norm_fin of the row's last pair (window 4j+3, finalized
            # during window 4j+4 groups 12/14).
            for jo in range(nsqb):
                for sc in range(jo * (sqb // P), (jo + 1) * (sqb // P)):
                    for db in range(D // 512):
                        items.append((
                            BIG,
                            min((4 * jo + 5) * nsk + 2, npairs * nsk + 8),
                            950,
                            lambda sc=sc, db=db: oproj_chunk(sc, db),
                        ))

            items.sort(key=lambda it: (it[0], it[1]))
            total_est = sum(it[2] for it in items)
            emitted_ns = 0.0

            def pump_fillers(slot, force_all=False):
                nonlocal emitted_ns
                target = (slot + 1) * total_est / total_slots
                while items:
                    k = None
                    for idx, it in enumerate(items):
                        if it[1] <= slot:
                            k = idx
                            break
                    if k is None:
                        return
                    dl = items[k][0]
                    if not (force_all or dl <= slot or emitted_ns < target):
                        return
                    it = items.pop(k)
                    it[3]()
                    emitted_ns += it[2]

            # ---------- pipelined attention ----------
            # window w: per group (key chunk) i the PE emits
            # [PV(w-1, i) both heads | fillers | QK-pair(w, i) + exp(w, i)]
            # (blocking QK last so the in-order PE queue never stalls), the
            # deferred bc/aT-mult of window w-2 at groups 7/11, and the DVE
            # reciprocal chains of window w-1 at window end.
            E_cur = {}
            pv_ps = {}
            norm_state = {}

            def emit_qk_exp(w, i):
                j, p = pairs[w]
                if i == 0:
                    E_cur[w] = ep.tile([P, nsk, 2, sqb], dt16, tag="E",
                                       name="E_t")
                E_t = E_cur[w]
                qkt = qkps.tile([P, 2, sqb], f32, tag="qk")
                js = slice(j * sqb, (j + 1) * sqb)
                # the pair's two heads contract disjoint row groups (64 each)
                # -> back-to-back matmuls run concurrently in the array.
                for hi in range(2):
                    po = DH * hi
                    nc.tensor.matmul(
                        qkt[:, hi, :],
                        lhsT=kT8[po:po + DH, p, i * P:(i + 1) * P],
                        rhs=qT8[po:po + DH, p, js],
                        start=True, stop=True,
                        tile_position=(po, 0),
                    )
                with nc.allow_low_precision(reason="fp16 probs by design"):
                    nc.scalar.activation(
                        out=E_t[:, i, :, :], in_=qkt,
                        func=AF.Exp, scale=scale,
                    )

            def emit_pv2(w, i):
                """PV for window w, key chunks 2i and 2i+1, both heads."""
                j, p = pairs[w]
                if i == 0:
                    pv_ps[w] = (
                        pvps.tile([P, sqb], f32, tag="pv", name="pv_a"),
                        pvps.tile([P, sqb], f32, tag="pv", name="pv_b"),
                    )
                E_t = E_cur[w]
                for u in (2 * i, 2 * i + 1):
                    for hi in range(2):
                        pv = pv_ps[w][hi]
                        nc.tensor.matmul(
                            pv[0:DH + 1, :],
                            lhsT=vpad[:, u, 2 * p + hi, :],
                            rhs=E_t[:, u, hi, :],
                            start=(u == 0),
                            stop=(u == nsk - 1),
                        )

            def norm_dve(w):
                pvs = pv_ps.pop(w)
                E_cur.pop(w)
                for hi in range(2):
                    pv = pvs[hi]
                    zsb = rcp.tile([1, sqb], f32, tag="zsb")
                    nc.vector.tensor_copy(out=zsb, in_=pv[DH:DH + 1, :])
                    zf = rcp.tile([1, sqb], f32, tag="zf")
                    nc.vector.reciprocal_approx_fast(out=zf, in_=zsb)
                    aun = rcp.tile([DH, sqb], dt16, tag="aun")
                    with nc.allow_low_precision(
                            reason="fp16 attn out by design"):
                        nc.vector.tensor_copy(out=aun, in_=pv[0:DH, :])
                        rc = rcp.tile([1, sqb], dt16, tag="rc")
                        nc.vector.tensor_copy(out=rc, in_=zf)
                    norm_state[(w, hi)] = (aun, rc)

            def norm_fin(w, hi):
                j, p = pairs[w]
                aun, rc = norm_state.pop((w, hi))
                js = slice(j * sqb, (j + 1) * sqb)
                bc = mmps.tile([P, sqb], f32, tag="mm")
                nc.tensor.matmul(
                    bc[0:DH, :], lhsT=ones_row, rhs=rc, start=True, stop=True,
                )
                with nc.allow_low_precision(reason="fp16 attn out by design"):
                    nc.vector.tensor_mul(
                        out=aT[DH * hi:DH * hi + DH, p, js],
                        in0=bc[0:DH, :],
                        in1=aun,
                    )

            # PV lags QK by ONE window, packed 2 chunks/group into groups
            # 0..7 so pv accumulation finishes mid-window: the reciprocal
            # chain runs at group 8 and the pv banks have half a window of
            # slack before their ring reuse at the next window's group 0.
            # The LAST pair's PV instead rides in its own window (groups
            # 9..15, chunks lagging exp by >=2 groups) so the epilogue only
            # owes 2 chunks + normalize + the last oproj row.
            for w in range(npairs + 1):
                for i in range(nsk):
                    slot = w * nsk + i
                    if 1 <= w < npairs and i < nsk // 2:
                        emit_pv2(w - 1, i)
                    if w == npairs - 1 and i >= 9:
                        emit_pv2(w, i - 9)          # chunks 0..13 of pair 15
                    if w == npairs and i < 2:
                        if i == 0:
                            emit_pv2(npairs - 1, 7)  # chunks 14,15
                        else:
                            norm_dve(npairs - 1)
                    if w >= 1 and i == 8 and w < npairs:
                        norm_dve(w - 1)
                    pump_fillers(slot)
                    if w >= 1 and w < npairs:
                        if i == 12:
                            norm_fin(w - 1, 0)
                        elif i == 14:
                            norm_fin(w - 1, 1)
                    elif w == npairs:
                        if i == 3:
                            norm_fin(w - 1, 0)
                        elif i == 5:
                            norm_fin(w - 1, 1)
                    if w < npairs:
                        emit_qk_exp(w, i)
            pump_fillers(BIG, force_all=True)
    nc.compile()
    return nc


def make_in_maps(inputs, s=S):
    """Host-side sharding/layout prep. Returns per-core input dicts."""
    import ml_dtypes

    Q, K, V = inputs["Q"], inputs["K"], inputs["V"]
    Wq, Wk, Wv, Wo = inputs["Wq"], inputs["Wk"], inputs["Wv"], inputs["Wo"]
    bq = inputs["bq"]

    f16 = np.float16
    f8 = ml_dtypes.float8_e4m3
    qdt = f8 if PROJ_FP8 else f16
    QT = np.ascontiguousarray(np.asarray(Q).transpose(0, 2, 1)).astype(qdt)
    KT = np.ascontiguousarray(np.asarray(K).transpose(0, 2, 1)).astype(qdt)
    VT = np.ascontiguousarray(np.asarray(V).transpose(0, 2, 1)).astype(f16)

    per_hg = []
    for hg in range(2):
        sl = slice(hg * DL, (hg + 1) * DL)
        per_hg.append({
            "WQT": np.ascontiguousarray(np.asarray(Wq)[sl, :].T).astype(qdt),
            "WKT": np.ascontiguousarray(np.asarray(Wk)[sl, :].T).astype(qdt),
            "WVT": np.ascontiguousarray(np.asarray(Wv)[sl, :].T).astype(f16),
            "WOT": np.ascontiguousarray(np.asarray(Wo)[:, sl].T).astype(f16),
            "BQ": np.ascontiguousarray(
                np.asarray(bq)[sl].reshape(CC, P).T
            ).astype(np.float32),
        })

    in_maps = []
    for core in range(N_CORES):
        b, hg = core // 2, core % 2
        m = {"QT": QT[b], "KT": KT[b], "VT": VT[b]}
        m.update(per_hg[hg])
        in_maps.append(m)
    return in_maps


def assemble_output(inputs, results):
    Wo, bv, bo = inputs["Wo"], inputs["bv"], inputs["bo"]
    extra = (np.asarray(bv, np.float32) @ np.asarray(Wo, np.float32).T
             + np.asarray(bo, np.float32))
    out = np.zeros((B, S, D), np.float32)
    for core in range(N_CORES):
        out[core // 2] += results[core]["OUT"].astype(np.float32)
    out += extra[None, None, :]
    return out


_NC_CACHE = {}


def _get_nc(s=S):
    if s not in _NC_CACHE:
        _NC_CACHE[s] = build_bass(s)
    return _NC_CACHE[s]


def _run(inputs, trace=False):
    from concourse.bass_utils import run_bass_kernel_spmd

    nc = _get_nc()
    in_maps = make_in_maps(inputs)
    res = run_bass_kernel_spmd(nc, in_maps, list(range(N_CORES)), trace=trace)
    return assemble_output(inputs, res.results), res


def kernel(**inputs):
    return _run(inputs, trace=False)[0]


def kernel_traced(**inputs):
    return _run(inputs, trace=True)


# revision 15
# speedup vs baseline: 1.0233x; 1.0233x over previous
"""Trainium2 Bass kernel for nn_MultiHeadAttention_57337813402001.

B=4, S=2048, D=1024, H=16 heads (DH=64). 8 NeuronCores.

Sharding: core = (batch b, head-group hg); hg splits the 16 heads into two
groups of 8 (tensor parallel on the QKV projection output columns and the
output projection input rows), b is data parallel. Each core computes a
partial output projection for its 8 heads; the host sums the two partials
per batch and adds the (algebraically folded) bias terms.

Algebraic simplifications (exact in real arithmetic):
  - bk drops out of softmax (adds a per-query constant to scores).
  - bv commutes through the attention average: folded into a host-side bias
    row bv @ Wo^T added at the end.
  - softmax without max-subtraction: |scores|/sqrt(d) < ~0.7 here.

v3 dataflow (vs the 407us v2):
  - QK^T head-PAIR packing: the two heads of a pair contract only DH=64
    rows each, so their score matmuls run CONCURRENTLY in the PE array via
    row tiling (tile_position (0,0) / (64,0), disjoint row groups) -- the
    pair costs ~1 matmul slot instead of 2.  Steps become 16 pair-windows
    (j-block x head-pair) of 16 single-chunk groups; scores for both heads
    of a key chunk land in one [128, 2(head), 512] PSUM tile so the exp op
    stays N=1024.
  - Q/K path in fp8e4: projections use MatmulPerfMode.DoubleRow; QK^T runs
    plain fp8 in the natural [pair-dh, s] layout. Scores only feed
    exp(s/32); measured ~4e-3 end to end.
  - V path and everything after exp stays fp16 (fp8 there costs ~4% output
    error).
  - Software-pipelined schedule per group: [PV(w-1) | fillers | QK(w)+exp]
    with the dependency-blocking QK last so the in-order PE queue never
    idles. Filler work (V/K/Q projection chunks, prior row's output
    projection) comes from a deadline + uniform-rate scheduler.
  - PSUM: qk 2x[128,2,512] + pv 2x[128,512] + mm(fillers+bc) 2x[128,512]
    = 8 banks exactly.
"""

import os
import sys

import numpy as np

for _p in ("/opt/trn_rl_repo",):
    if _p not in sys.path and os.path.isdir(_p):
        sys.path.insert(0, _p)

B, S, D, H = 4, 2048, 1024, 16
DH = D // H          # 64
HL = H // 2          # 8 heads per core
DL = HL * DH         # 512 local hidden
P = 128
KC = D // P          # 8 d_in chunks
CC = DL // P         # 4 local d_out chunks (= head pairs)
N_CORES = 8

QK_FP8 = True        # fp8 for QK^T scores
PROJ_FP8 = True      # DoubleRow fp8 for the Q/K projections (host fp8 in)


def build_bass(s=S):
    import concourse.bass as bass  # noqa: F401
    import concourse.mybir as mybir
    from concourse import bacc
    from concourse.tile import TileContext

    dt16 = mybir.dt.float16
    f8 = mybir.dt.float8e4
    f32 = mybir.dt.float32
    AF = mybir.ActivationFunctionType
    DR = mybir.MatmulPerfMode.DoubleRow

    nsk = s // P                 # key chunks (16) = groups per window
    sqb = min(512, s)            # sq block
    nsqb = s // sqb              # 4 j rows
    sb_blk = min(512, s)
    nsb = s // sb_blk            # 4 projection s blocks
    qdt = f8 if PROJ_FP8 else dt16

    nc = bacc.Bacc()
    QT = nc.declare_dram_parameter("QT", [D, s], qdt, isOutput=False)
    KT = nc.declare_dram_parameter("KT", [D, s], qdt, isOutput=False)
    VT = nc.declare_dram_parameter("VT", [D, s], dt16, isOutput=False)
    WQT = nc.declare_dram_parameter("WQT", [D, DL], qdt, isOutput=False)
    WKT = nc.declare_dram_parameter("WKT", [D, DL], qdt, isOutput=False)
    WVT = nc.declare_dram_parameter("WVT", [D, DL], dt16, isOutput=False)
    WOT = nc.declare_dram_parameter("WOT", [DL, D], dt16, isOutput=False)
    BQ = nc.declare_dram_parameter("BQ", [P, CC], f32, isOutput=False)
    OUT = nc.declare_dram_parameter("OUT", [s, D], dt16, isOutput=True)

    with TileContext(nc) as tc:
        with (
            tc.tile_pool(name="w", bufs=1) as wp,
            tc.tile_pool(name="stq", bufs=1) as stq,
            tc.tile_pool(name="stk", bufs=1) as stk,
            tc.tile_pool(name="stv", bufs=2) as stv,
            tc.tile_pool(name="qkv", bufs=1) as qkvp,
            tc.tile_pool(name="E", bufs=2) as ep,
            tc.tile_pool(name="rc", bufs=2) as rcp,
            tc.tile_pool(name="ost", bufs=3) as ostp,
            tc.tile_pool(name="qkps", bufs=2, space="PSUM") as qkps,
            tc.tile_pool(name="pvps", bufs=2, space="PSUM") as pvps,
            tc.tile_pool(name="mmps", bufs=2, space="PSUM") as mmps,
        ):
            # --- weights / constants ---
            wq = wp.tile([P, KC, DL], qdt, tag="wq")
            wk = wp.tile([P, KC, DL], qdt, tag="wk")
            wv = wp.tile([P, KC, DL], dt16, tag="wv")
            wo = wp.tile([P, CC, D], dt16, tag="wo")
            bq = wp.tile([P, CC], f32, tag="bq")
            ones_row = wp.tile([1, DH], dt16, tag="ones")
            qst = stq.tile([P, KC, s], qdt, tag="qst")
            kst = stk.tile([P, KC, s], qdt, tag="kst")

            # DMA issue order feeds the critical path: k projection of
            # s-block b needs only wk + kst block b; q pair0 needs wq + qst
            # block 0. Everything else follows. (All staging on the SP
            # hardware DGE queue: the gpsimd queue is software-DGE and its
            # ~1.1us triggers delayed staging when tried.)
            def stage_blk(dst, src, blk, eng):
                eng.dma_start(
                    dst[:, :, blk * sb_blk:(blk + 1) * sb_blk],
                    src[:, blk * sb_blk:(blk + 1) * sb_blk].rearrange(
                        "(kc p) ss -> p kc ss", p=P
                    ),
                )

            nc.sync.dma_start(wk, WKT[:].rearrange("(kc p) m -> p kc m", p=P))
            stage_blk(kst, KT, 0, nc.sync)
            nc.sync.dma_start(wq, WQT[:].rearrange("(kc p) m -> p kc m", p=P))
            nc.sync.dma_start(bq, BQ[:])
            stage_blk(qst, QT, 0, nc.sync)
            for blk in range(1, nsb):
                stage_blk(kst, KT, blk, nc.sync)
            nc.sync.dma_start(wv, WVT[:].rearrange("(kc p) m -> p kc m", p=P))
            for blk in range(1, nsb):
                stage_blk(qst, QT, blk, nc.sync)
            nc.sync.dma_start(wo, WOT[:].rearrange("(cc p) m -> p cc m", p=P))
            nc.vector.memset(ones_row, 1.0)

            # q/k fp8 in natural projection layout: [128, pair, s], partition
            # = 64*hi + dh for head 2*pair + hi. QK^T runs plain fp8 (K=64)
            # with the pair's two heads row-tiled concurrently.
            qT8 = qkvp.tile([P, CC, s], f8, tag="qT8")
            kT8 = qkvp.tile([P, CC, s], f8, tag="kT8")
            vpad = qkvp.tile([P, nsk, HL, DH + 1], dt16, tag="vpad")
            aT = qkvp.tile([P, CC, s], dt16, tag="aT")
            nc.vector.memset(vpad[:, :, :, DH], 1.0)

            scale = 1.0 / np.sqrt(np.float32(D)).item()

            # ---------- emission helpers ----------
            def proj_qk_chunk(xst, w, dst8, c, blk, bias=None):
                """One [128,512] chunk of a q/k projection (fp8 DoubleRow,
                2 d_in k-tiles per matmul), cast straight into dst8[:, c]."""
                ps = mmps.tile([P, sb_blk], f32, tag="mm")
                if PROJ_FP8:
                    for k2 in range(KC // 2):
                        nc.tensor.matmul(
                            ps,
                            lhsT=w[:, 2 * k2:2 * k2 + 2, c * P:(c + 1) * P],
                            rhs=xst[:, 2 * k2:2 * k2 + 2,
                                    blk * sb_blk:(blk + 1) * sb_blk],
                            start=(k2 == 0),
                            stop=(k2 == KC // 2 - 1),
                            perf_mode=DR,
                        )
                else:
                    for k in range(KC):
                        nc.tensor.matmul(
                            ps,
                            lhsT=w[:, k, c * P:(c + 1) * P],
                            rhs=xst[:, k, blk * sb_blk:(blk + 1) * sb_blk],
                            start=(k == 0),
                            stop=(k == KC - 1),
                        )
                dsl = dst8[:, c, blk * sb_blk:(blk + 1) * sb_blk]
                with nc.allow_low_precision(reason="fp8 q/k by design"):
                    if bias is not None:
                        nc.vector.tensor_scalar_add(
                            out=dsl, in0=ps, scalar1=bias[:, c:c + 1],
                        )
                    else:
                        nc.vector.tensor_copy(out=dsl, in_=ps)

            def v_chunk(xv, cg, li, gi):
                """V projection for pair-group cg (pairs 2cg,2cg+1), local
                chunk li of the staged block = global sk chunk gi; N=256."""
                ps = mmps.tile([P, 256], f32, tag="mm")
                for k in range(KC):
                    nc.tensor.matmul(
                        ps,
                        lhsT=xv[:, k, li * P:(li + 1) * P],
                        rhs=wv[:, k, cg * 256:(cg + 1) * 256],
                        start=(k == 0),
                        stop=(k == KC - 1),
                    )
                with nc.allow_low_precision(reason="fp16 v by design"):
                    nc.vector.tensor_copy(
                        out=vpad[:, gi, 4 * cg:4 * cg + 4, 0:DH],
                        in_=ps.rearrange("p (h d) -> p h d", d=DH),
                    )

            # ---------- filler queue (PE work interleaved into the
            # attention pipeline; ~each item <= ~1us of PE time) ----------
            def stage_v(blk):
                xv = stv.tile([P, KC, sb_blk], dt16, tag="stv")
                nc.sync.dma_start(
                    xv,
                    VT[:, blk * sb_blk:(blk + 1) * sb_blk].rearrange(
                        "(kc p) ss -> p kc ss", p=P
                    ),
                )
                return xv

            def oproj_chunk(sc, db):
                ps = mmps.tile([P, 512], f32, tag="mm")
                for c in range(CC):
                    nc.tensor.matmul(
                        ps,
                        lhsT=aT[:, c, sc * P:(sc + 1) * P],
                        rhs=wo[:, c, db * 512:(db + 1) * 512],
                        start=(c == 0),
                        stop=(c == CC - 1),
                    )
                ot = ostp.tile([P, 512], dt16, tag="ost")
                with nc.allow_low_precision(reason="fp16 partial"):
                    nc.vector.tensor_copy(out=ot, in_=ps)
                nc.sync.dma_start(
                    OUT[sc * P:(sc + 1) * P, db * 512:(db + 1) * 512], ot
                )

            # ---------- preamble: only k c0 blk0 + q pair0 c0 gate QK(0,0);
            # the rest become tight-deadline fillers.
            proj_qk_chunk(kst, wk, kT8, 0, 0)
            proj_qk_chunk(qst, wq, qT8, 0, 0, bias=bq)

            # ---------- filler scheduler ----------
            # Items = (deadline_slot, release_slot, est_pe_ns, fn), emitted
            # into group slots at a uniform PE-time rate with deadline
            # forcing, so the in-order PE queue always has ready work (the
            # p-state model halves the PE clock for 3us after any idle gap).
            # slot = window * nsk + group; windows = npairs + 2.
            pairs = [(j, p) for j in range(nsqb) for p in range(CC)]
            npairs = len(pairs)                      # 16
            total_slots = (npairs + 1) * nsk
            BIG = 10 ** 9
            items = []
            v_stage = {}

            def v_item(cg, blk, li):
                def _f():
                    if blk not in v_stage or v_stage[blk][1] != (cg,):
                        v_stage[blk] = (stage_v(blk), (cg,))
                    v_chunk(v_stage[blk][0], cg, li, blk * (sb_blk // P) + li)
                gi = blk * (sb_blk // P) + li
                # PV(w-1) in window w reads vpad chunk gi at group gi//2;
                # cg first used by pair 2cg (window 2cg, PV in 2cg+1).
                dl = (1 + 2 * cg) * nsk + gi // 2 - 2
                return (max(1, dl), 0, 900, _f)

            for cg in range(2):
                for blk in range(nsb):
                    for li in range(sb_blk // P):
                        items.append(v_item(cg, blk, li))

            for c in range(CC):
                for blk in range(nsb):
                    if c == 0 and blk == 0:
                        continue
                    # kT8[:, c, blk] feeds QK of pair-window c, groups
                    # 4blk..4blk+3; release keeps early c0 blocks from
                    # popping before their staging DMAs land.
                    items.append((
                        max(1, c * nsk + 4 * blk - 4),
                        max(0, 4 * blk - 3) if c == 0 else 0, 900,
                        lambda c=c, blk=blk: proj_qk_chunk(kst, wk, kT8, c, blk),
                    ))
            for jq in range(nsqb):
                for c in range(CC):
                    if jq == 0 and c == 0:
                        continue
                    # qT8[:, c, jq] feeds QK of window 4jq + c, group 0.
                    items.append((
                        max(1, (4 * jq + c) * nsk - 12), 0, 900,
                        lambda c=c, jq=jq: proj_qk_chunk(
                            qst, wq, qT8, c, jq, bias=bq),
                    ))
            # output projection of row j: released one window after
            # norm_fin of the row's last pair (window 4j+3, finalized
            # during window 4j+4 groups 12/14).
            for jo in range(nsqb):
                for sc in range(jo * (sqb // P), (jo + 1) * (sqb // P)):
                    for db in range(D // 512):
                        items.append((
                            BIG,
                            min((4 * jo + 5) * nsk + 2, npairs * nsk + 8),
                            950,
                            lambda sc=sc, db=db: oproj_chunk(sc, db),
                        ))

            items.sort(key=lambda it: (it[0], it[1]))
            total_est = sum(it[2] for it in items)
            emitted_ns = 0.0

            def pump_fillers(slot, force_all=False):
                nonlocal emitted_ns
                target = (slot + 1) * total_est / total_slots
                while items:
                    k = None
                    for idx, it in enumerate(items):
                        if it[1] <= slot:
                            k = idx
                            break
                    if k is None:
                        return
                    dl = items[k][0]
                    if not (force_all or dl <= slot or emitted_ns < target):
                        return
                    it = items.pop(k)
                    it[3]()
                    emitted_ns += it[2]

            # ---------- pipelined attention ----------
            # window w: per group (key chunk) i the PE emits
            # [PV(w-1, i) both heads | fillers | QK-pair(w, i) + exp(w, i)]
            # (blocking QK last so the in-order PE queue never stalls), the
            # deferred bc/aT-mult of window w-2 at groups 7/11, and the DVE
            # reciprocal chains of window w-1 at window end.
            E_cur = {}
            pv_ps = {}
            norm_state = {}

            def emit_qk_exp(w, i):
                j, p = pairs[w]
                if i == 0:
                    E_cur[w] = ep.tile([P, nsk, 2, sqb], dt16, tag="E",
                                       name="E_t")
                E_t = E_cur[w]
                qkt = qkps.tile([P, 2, sqb], f32, tag="qk")
                js = slice(j * sqb, (j + 1) * sqb)
                # the pair's two heads contract disjoint row groups (64 each)
                # -> back-to-back matmuls run concurrently in the array.
                for hi in range(2):
                    po = DH * hi
                    nc.tensor.matmul(
                        qkt[:, hi, :],
                        lhsT=kT8[po:po + DH, p, i * P:(i + 1) * P],
                        rhs=qT8[po:po + DH, p, js],
                        start=True, stop=True,
                        tile_position=(po, 0),
                    )
                with nc.allow_low_precision(reason="fp16 probs by design"):
                    nc.scalar.activation(
                        out=E_t[:, i, :, :], in_=qkt,
                        func=AF.Exp, scale=scale,
                    )

            def emit_pv2(w, i):
                """PV for window w, key chunks 2i and 2i+1, both heads."""
                j, p = pairs[w]
                if i == 0:
                    pv_ps[w] = (
                        pvps.tile([P, sqb], f32, tag="pv", name="pv_a"),
                        pvps.tile([P, sqb], f32, tag="pv", name="pv_b"),
                    )
                E_t = E_cur[w]
                for u in (2 * i, 2 * i + 1):
                    for hi in range(2):
                        pv = pv_ps[w][hi]
                        nc.tensor.matmul(
                            pv[0:DH + 1, :],
                            lhsT=vpad[:, u, 2 * p + hi, :],
                            rhs=E_t[:, u, hi, :],
                            start=(u == 0),
                            stop=(u == nsk - 1),
                        )

            def norm_dve(w):
                pvs = pv_ps.pop(w)
                E_cur.pop(w)
                for hi in range(2):
                    pv = pvs[hi]
                    zsb = rcp.tile([1, sqb], f32, tag="zsb")
                    nc.vector.tensor_copy(out=zsb, in_=pv[DH:DH + 1, :])
                    zf = rcp.tile([1, sqb], f32, tag="zf")
                    nc.vector.reciprocal_approx_fast(out=zf, in_=zsb)
                    aun = rcp.tile([DH, sqb], dt16, tag="aun")
                    with nc.allow_low_precision(
                            reason="fp16 attn out by design"):
                        nc.vector.tensor_copy(out=aun, in_=pv[0:DH, :])
                        rc = rcp.tile([1, sqb], dt16, tag="rc")
                        nc.vector.tensor_copy(out=rc, in_=zf)
                    norm_state[(w, hi)] = (aun, rc)

            def norm_fin(w, hi):
                j, p = pairs[w]
                aun, rc = norm_state.pop((w, hi))
                js = slice(j * sqb, (j + 1) * sqb)
                bc = mmps.tile([P, sqb], f32, tag="mm")
                nc.tensor.matmul(
                    bc[0:DH, :], lhsT=ones_row, rhs=rc, start=True, stop=True,
                )
                with nc.allow_low_precision(reason="fp16 attn out by design"):
                    nc.vector.tensor_mul(
                        out=aT[DH * hi:DH * hi + DH, p, js],
                        in0=bc[0:DH, :],
                        in1=aun,
                    )

            # PV lags QK by ONE window, packed 2 chunks/group into groups
            # 0..7 so pv accumulation finishes mid-window: the reciprocal
            # chain runs at group 8 and the pv banks have half a window of
            # slack before their ring reuse at the next window's group 0.
            # The LAST pair's PV instead rides in its own window (groups
            # 9..15, chunks lagging exp by >=2 groups) so the epilogue only
            # owes 2 chunks + normalize + the last oproj row.
            for w in range(npairs + 1):
                for i in range(nsk):
                    slot = w * nsk + i
                    if 1 <= w < npairs and i < nsk // 2:
                        emit_pv2(w - 1, i)
                    if w == npairs - 1 and i >= 9:
                        emit_pv2(w, i - 9)          # chunks 0..13 of pair 15
                    if w == npairs and i < 2:
                        if i == 0:
                            emit_pv2(npairs - 1, 7)  # chunks 14,15
                        else:
                            norm_dve(npairs - 1)
                    if w >= 1 and i == 8 and w < npairs:
                        norm_dve(w - 1)
                    pump_fillers(slot)
                    if w >= 1 and w < npairs:
                        if i == 12:
                            norm_fin(w - 1, 0)
                        elif i == 14:
                            norm_fin(w - 1, 1)
                    elif w == npairs:
                        if i == 3:
                            norm_fin(w - 1, 0)
                        elif i == 5:
                            norm_fin(w - 1, 1)
                    if w < npairs:
                        emit_qk_exp(w, i)
            pump_fillers(BIG, force_all=True)
    nc.compile()
    return nc


def make_in_maps(inputs, s=S):
    """Host-side sharding/layout prep. Returns per-core input dicts."""
    import ml_dtypes

    Q, K, V = inputs["Q"], inputs["K"], inputs["V"]
    Wq, Wk, Wv, Wo = inputs["Wq"], inputs["Wk"], inputs["Wv"], inputs["Wo"]
    bq = inputs["bq"]

    f16 = np.float16
    f8 = ml_dtypes.float8_e4m3
    qdt = f8 if PROJ_FP8 else f16
    QT = np.ascontiguousarray(np.asarray(Q).transpose(0, 2, 1)).astype(qdt)
    KT = np.ascontiguousarray(np.asarray(K).transpose(0, 2, 1)).astype(qdt)
    VT = np.ascontiguousarray(np.asarray(V).transpose(0, 2, 1)).astype(f16)

    per_hg = []
    for hg in range(2):
        sl = slice(hg * DL, (hg + 1) * DL)
        per_hg.append({
            "WQT": np.ascontiguousarray(np.asarray(Wq)[sl, :].T).astype(qdt),
            "WKT": np.ascontiguousarray(np.asarray(Wk)[sl, :].T).astype(qdt),
            "WVT": np.ascontiguousarray(np.asarray(Wv)[sl, :].T).astype(f16),
            "WOT": np.ascontiguousarray(np.asarray(Wo)[:, sl].T).astype(f16),
            "BQ": np.ascontiguousarray(
                np.asarray(bq)[sl].reshape(CC, P).T
            ).astype(np.float32),
        })

    in_maps = []
    for core in range(N_CORES):
        b, hg = core // 2, core % 2
        m = {"QT": QT[b], "KT": KT[b], "VT": VT[b]}
        m.update(per_hg[hg])
        in_maps.append(m)
    return in_maps


def assemble_output(inputs, results):
    Wo, bv, bo = inputs["Wo"], inputs["bv"], inputs["bo"]
    extra = (np.asarray(bv, np.float32) @ np.asarray(Wo, np.float32).T
             + np.asarray(bo, np.float32))
    out = np.zeros((B, S, D), np.float32)
    for core in range(N_CORES):
        out[core // 2] += results[core]["OUT"].astype(np.float32)
    out += extra[None, None, :]
    return out


_NC_CACHE = {}


def _get_nc(s=S):
    if s not in _NC_CACHE:
        _NC_CACHE[s] = build_bass(s)
    return _NC_CACHE[s]


def _run(inputs, trace=False):
    from concourse.bass_utils import run_bass_kernel_spmd

    nc = _get_nc()
    in_maps = make_in_maps(inputs)
    res = run_bass_kernel_spmd(nc, in_maps, list(range(N_CORES)), trace=trace)
    return assemble_output(inputs, res.results), res


def kernel(**inputs):
    return _run(inputs, trace=False)[0]


def kernel_traced(**inputs):
    return _run(inputs, trace=True)
